# revision 24
# baseline (speedup 1.0000x reference)
"""Trainium2 Bass kernel for nn_GSPolicyNetLSTM (Gumbel-softmax policy net).

Device strategy (unchanged from the correct baseline):
  - The sender/receiver LSTM decode is tiny and fully serial -> replicate it
    on every core (identical compute, no communication).
  - The huge output projection W_r [262144, 100] is sharded row-wise across
    the 8 cores ([32768, 100] each, + bias row).  Its DMA streams in the
    background while the LSTM decode runs.
  - Each core returns exp(logits_shard) [128, 256] plus the shard sum; the
    host concatenates and divides by the global sum (softmax denominator).

Dispatch strategy (the actual speedup vs. the baseline):
  run_bass_kernel_spmd -> run_bass_via_pjrt rebuilds its jit closure on
  every call, so each call pays a fresh JAX trace + XLA compile + NEFF
  cache lookup, re-concatenates ~150 MB of replicated weights on the host,
  and re-ships them over the axon tunnel.  Here we build the identical
  jit(shard_map(bass_exec)) wrapper ONCE, keep every weight-derived tensor
  device-resident, and per call only upload x (16 KB) and gumbel (4.8 KB)
  before dispatching the cached executable.

Key simplifications (exact in forward pass):
  - st = hard + soft - stop_grad(soft) == hard  -> emitted symbols are exact
    one-hots, so W_ih @ sym is a column gather (register-indexed slice).
  - Post-EOS sender state is dead (receiver updates are gated by `valid`),
    so sender h/c/idx never need freezing.
  - At t==29 the reference emits the EOS constant, so the receiver's last
    input is the static EOS column.
"""

import numpy as np

IN_SHAPE = 4096
H_S = 250
H_R = 100
MAX_LEN = 30
VOCAB = 40
OUT_CLS = 512 ** 2
N_CORES = 8
SHARD = OUT_CLS // N_CORES          # 32768
NJ = SHARD // 128                   # 256 matmul column-tiles per core

_CACHE = {}

_W_KEYS = ("W_s1", "b_s1", "W_ih1", "W_hh1", "b_ih1", "b_hh1", "W_p", "b_p",
           "W_ih2", "W_hh2", "b_ih2", "b_hh2", "W_r", "b_r")


def _build_program(f16_out=True):
    """f16_out=True: fast-path program (fp16 out_exp, half the fetch bytes).
    f16_out=False: byte-identical to the original baseline program, used by
    the run_bass_kernel_spmd fallback path (its jit module name 'jit__body'
    may resolve to the staged baseline executable, which expects f32)."""
    import concourse.bacc as bacc
    import concourse.bass as bass
    import concourse.mybir as mybir
    import concourse.tile as tile

    f32 = mybir.dt.float32
    u32 = mybir.dt.uint32
    AF = mybir.ActivationFunctionType
    ALU = mybir.AluOpType

    nc = bacc.Bacc("TRN2", target_bir_lowering=False, debug=False,
                   num_devices=N_CORES)

    # ---- DRAM I/O ----
    d_xT = nc.dram_tensor("xT", [128, 32], f32, kind="ExternalInput")
    d_ws1t = nc.dram_tensor("ws1t", [4096, 250], f32, kind="ExternalInput")
    d_bs1 = nc.dram_tensor("bs1", [1, 250], f32, kind="ExternalInput")
    d_whh1t = nc.dram_tensor("whh1t", [250, 1024], f32, kind="ExternalInput")
    d_wih1g = nc.dram_tensor("wih1g", [128, 41 * 8], f32, kind="ExternalInput")
    d_wpt = nc.dram_tensor("wpt", [250, 40], f32, kind="ExternalInput")
    d_gz = nc.dram_tensor("gz", [1, MAX_LEN * 40], f32, kind="ExternalInput")
    d_whh2t = nc.dram_tensor("whh2t", [100, 400], f32, kind="ExternalInput")
    d_wih2g = nc.dram_tensor("wih2g", [100, 41 * 4], f32, kind="ExternalInput")
    d_wrt = nc.dram_tensor("wrt", [101, SHARD], f32, kind="ExternalInput")
    f16 = mybir.dt.float16
    # fp16 output halves the device->host payload over the ~75ms-RTT axon
    # tunnel; exp(logits) is O(1) here so fp16's 2^-11 rel error is far
    # inside the 2e-2 gate.
    d_oexp = nc.dram_tensor("out_exp", [128, NJ],
                            f16 if f16_out else f32, kind="ExternalOutput")
    d_osum = nc.dram_tensor("out_s", [1, 1], f32, kind="ExternalOutput")

    with tile.TileContext(nc) as tc:
        with (
            tc.tile_pool(name="const", bufs=1) as cpool,
            tc.tile_pool(name="state", bufs=1) as spool,
            tc.tile_pool(name="tmp", bufs=3) as tpool,
        ):
            # ---- SBUF residents ----
            xT = cpool.tile([128, 32], f32, tag="xT")
            ws1t = cpool.tile([128, 32, 250], f32, tag="ws1t")
            bs1 = cpool.tile([1, 250], f32, tag="bs1")
            whh1a = cpool.tile([128, 1024], f32, tag="whh1a")
            whh1b = cpool.tile([122, 1024], f32, tag="whh1b")
            wih1g = cpool.tile([128, 41 * 8], f32, tag="wih1g")
            wpta = cpool.tile([128, 40], f32, tag="wpta")
            wptb = cpool.tile([122, 40], f32, tag="wptb")
            gz = cpool.tile([1, MAX_LEN * 40], f32, tag="gz")
            whh2t = cpool.tile([100, 400], f32, tag="whh2t")
            wih2g = cpool.tile([100, 41 * 4], f32, tag="wih2g")
            wrt = cpool.tile([101, SHARD], f32, tag="wrt")
            ones_row = cpool.tile([1, 128], f32, tag="ones_row")
            one1 = cpool.tile([1, 1], f32, tag="one1")
            ones_col = cpool.tile([128, 1], f32, tag="ones_col")

            h_a = spool.tile([128, 1], f32, tag="h_a")
            h_b = spool.tile([122, 1], f32, tag="h_b")
            c_st = spool.tile([128, 2], f32, tag="c_st")
            done = spool.tile([128, 1], f32, tag="done")
            h2 = spool.tile([100, 1], f32, tag="h2")
            c2 = spool.tile([100, 1], f32, tag="c2")
            h2aug = spool.tile([101, 1], f32, tag="h2aug")
            h0row = spool.tile([1, 256], f32, tag="h0row")

            # ---- input DMAs (small first, big W_r last so it streams in
            # the background on the same HWDGE FIFO) ----
            nc.sync.dma_start(xT[:], d_xT[:])
            nc.sync.dma_start(bs1[:], d_bs1[:])
            nc.sync.dma_start(whh1a[:], d_whh1t[0:128, :])
            nc.sync.dma_start(whh1b[:], d_whh1t[128:250, :])
            nc.sync.dma_start(wih1g[:], d_wih1g[:])
            nc.sync.dma_start(wpta[:], d_wpt[0:128, :])
            nc.sync.dma_start(wptb[:], d_wpt[128:250, :])
            nc.sync.dma_start(gz[:], d_gz[:])
            nc.sync.dma_start(whh2t[:], d_whh2t[:])
            nc.sync.dma_start(wih2g[:], d_wih2g[:])
            ws1t_re = d_ws1t.ap().rearrange("(c p) j -> p c j", p=128)
            for cc in range(4):
                nc.sync.dma_start(ws1t[:, 8 * cc:8 * cc + 8, :],
                                  ws1t_re[:, 8 * cc:8 * cc + 8, :])
            nc.sync.dma_start(wrt[:], d_wrt[:])

            # ---- constants / state init ----
            nc.vector.memset(ones_row[:], 1.0)
            nc.vector.memset(one1[:], 1.0)
            nc.vector.memset(ones_col[:], 1.0)
            nc.vector.memset(done[:], 0.0)
            nc.vector.memset(h2[:], 0.0)
            nc.vector.memset(c2[:], 0.0)
            nc.vector.memset(c_st[:], 0.0)
            nc.vector.memset(h2aug[:], 1.0)  # row 100 stays 1.0 (bias)

            # ---- phase 1: h0 = relu(W_s1 @ x + b_s1) ----
            with tc.tile_pool(name="ph0", bufs=2,
                              space=bass.MemorySpace.PSUM) as p0:
                h0ps = p0.tile([1, 256], f32, tag="h0ps")
                nc.tensor.matmul(h0ps[0:1, 0:250], one1[:], bs1[:],
                                 start=True, stop=False)
                for c in range(32):
                    nc.tensor.matmul(h0ps[0:1, 0:250], xT[:, c:c + 1],
                                     ws1t[:, c, :], start=False,
                                     stop=(c == 31))
                nc.scalar.activation(h0row[0:1, 0:250], h0ps[0:1, 0:250],
                                     AF.Relu)
                tra = p0.tile([128, 1], f32, tag="trps")
                nc.tensor.matmul(tra[:], h0row[0:1, 0:128], one1[:],
                                 start=True, stop=True)
                nc.vector.tensor_copy(h_a[:], tra[:])
                trb = p0.tile([128, 1], f32, tag="trps")
                nc.tensor.matmul(trb[0:122, :], h0row[0:1, 128:250], one1[:],
                                 start=True, stop=True)
                nc.vector.tensor_copy(h_b[:], trb[0:122, :])

            # ---- phase 2: decode loop ----
            with (
                tc.tile_pool(name="psA", bufs=2,
                             space=bass.MemorySpace.PSUM) as psA,
                tc.tile_pool(name="psZ", bufs=2,
                             space=bass.MemorySpace.PSUM) as psZ,
                tc.tile_pool(name="psE", bufs=1,
                             space=bass.MemorySpace.PSUM) as psE,
                tc.tile_pool(name="psG2", bufs=2,
                             space=bass.MemorySpace.PSUM) as psG2,
            ):
                rv = None  # ScalarValue of previous step's argmax index
                for t in range(MAX_LEN):
                    # valid_t = 1 - done  (pre-update)
                    vbc = tpool.tile([128, 1], f32, tag="vbc")
                    nc.vector.tensor_scalar(vbc[:], done[:], -1.0, 1.0,
                                            ALU.mult, ALU.add)

                    if t < MAX_LEN - 1:
                        # -- sender LSTM cell --
                        g1 = psA.tile([128, 8], f32, tag="g1")
                        for jj in range(8):
                            nc.tensor.matmul(
                                g1[:, jj:jj + 1],
                                whh1a[:, 128 * jj:128 * (jj + 1)], h_a[:],
                                start=True, stop=False)
                            nc.tensor.matmul(
                                g1[:, jj:jj + 1],
                                whh1b[:, 128 * jj:128 * (jj + 1)], h_b[:],
                                start=False, stop=True)
                        if t == 0:
                            ihsl = wih1g[:, 40 * 8:41 * 8]   # SOS slot
                        else:
                            ihsl = wih1g[:, bass.ts(rv, 8)]
                        gsum = tpool.tile([128, 8], f32, tag="gsum")
                        nc.vector.tensor_add(gsum[:], g1[:], ihsl)
                        act = tpool.tile([128, 8], f32, tag="act")
                        nc.scalar.activation(act[:, 0:6], gsum[:, 0:6],
                                             AF.Sigmoid)
                        nc.scalar.activation(act[:, 6:8], gsum[:, 6:8],
                                             AF.Tanh)
                        fc = tpool.tile([128, 2], f32, tag="fc")
                        nc.vector.tensor_mul(fc[:], act[:, 2:4], c_st[:])
                        ig = tpool.tile([128, 2], f32, tag="ig")
                        nc.vector.tensor_mul(ig[:], act[:, 0:2], act[:, 6:8])
                        nc.vector.tensor_add(c_st[:], fc[:], ig[:])
                        tch = tpool.tile([128, 2], f32, tag="tch")
                        nc.scalar.activation(tch[:], c_st[:], AF.Tanh)
                        nc.vector.tensor_mul(h_a[:], act[:, 4:5],
                                             tch[:, 0:1])
                        nc.vector.tensor_mul(h_b[:], act[0:122, 5:6],
                                             tch[0:122, 1:2])

                        # -- logits + gumbel + argmax --
                        zps = psZ.tile([1, 40], f32, tag="zps")
                        nc.tensor.matmul(zps[:], h_a[:], wpta[:],
                                         start=True, stop=False)
                        nc.tensor.matmul(zps[:], h_b[:], wptb[:],
                                         start=False, stop=True)
                        zsb = tpool.tile([1, 40], f32, tag="zsb")
                        nc.vector.tensor_add(zsb[:], zps[:],
                                             gz[0:1, 40 * t:40 * (t + 1)])
                        mx8 = tpool.tile([1, 8], f32, tag="mx8")
                        nc.vector.max(mx8[:], zsb[:])
                        idx8 = tpool.tile([1, 8], u32, tag="idx8")
                        nc.vector.max_index(idx8[:], mx8[:], zsb[:])
                        reg = nc.alloc_register(mybir.EngineType.DVE,
                                                f"ridx{t}")
                        nc.vector.reg_load(reg, idx8[0:1, 0:1])
                        rv = nc.snap(reg, donate=True, min_val=0,
                                     max_val=VOCAB - 1)

                        # -- done |= (z[eos] == max) broadcast --
                        eos1 = tpool.tile([1, 1], f32, tag="eos1")
                        nc.vector.tensor_scalar(eos1[:], zsb[0:1, 39:40],
                                                mx8[0:1, 0:1], None,
                                                ALU.is_equal)
                        ebc = psE.tile([128, 1], f32, tag="ebc")
                        nc.tensor.matmul(ebc[:], ones_row[:], eos1[:],
                                         start=True, stop=True)
                        nc.vector.tensor_max(done[:], done[:], ebc[:])

                    # -- receiver LSTM cell (input: msg_t one-hot) --
                    g2 = psG2.tile([100, 4], f32, tag="g2")
                    for g in range(4):
                        nc.tensor.matmul(g2[:, g:g + 1],
                                         whh2t[:, 100 * g:100 * (g + 1)],
                                         h2[:], start=True, stop=True)
                    if t == MAX_LEN - 1:
                        ihsl2 = wih2g[:, 39 * 4:40 * 4]  # forced EOS
                    else:
                        ihsl2 = wih2g[:, bass.ts(rv, 4)]
                    gsum2 = tpool.tile([100, 4], f32, tag="gsum2")
                    nc.vector.tensor_add(gsum2[:], g2[:], ihsl2)
                    act2 = tpool.tile([100, 4], f32, tag="act2")
                    nc.scalar.activation(act2[:, 0:3], gsum2[:, 0:3],
                                         AF.Sigmoid)
                    nc.scalar.activation(act2[:, 3:4], gsum2[:, 3:4],
                                         AF.Tanh)
                    fc2 = tpool.tile([100, 1], f32, tag="fc2")
                    nc.vector.tensor_mul(fc2[:], act2[:, 1:2], c2[:])
                    ig2 = tpool.tile([100, 1], f32, tag="ig2")
                    nc.vector.tensor_mul(ig2[:], act2[:, 0:1], act2[:, 3:4])
                    c2n = tpool.tile([100, 1], f32, tag="c2n")
                    nc.vector.tensor_add(c2n[:], fc2[:], ig2[:])
                    tc2 = tpool.tile([100, 1], f32, tag="tc2")
                    nc.scalar.activation(tc2[:], c2n[:], AF.Tanh)
                    h2n = tpool.tile([100, 1], f32, tag="h2n")
                    nc.vector.tensor_mul(h2n[:], act2[:, 2:3], tc2[:])
                    vmask = vbc[0:100, :].bitcast(mybir.dt.int32)
                    nc.vector.copy_predicated(c2[:], vmask, c2n[:])
                    nc.vector.copy_predicated(h2[:], vmask, h2n[:])

            # ---- phase 3: logits shard = W_r @ hR + b_r; exp + sum ----
            nc.vector.tensor_copy(h2aug[0:100, :], h2[:])
            with tc.tile_pool(name="psW", bufs=1,
                              space=bass.MemorySpace.PSUM) as psW:
                lg = psW.tile([128, NJ], f32, tag="lg")
                for j in range(NJ):
                    nc.tensor.matmul(lg[:, j:j + 1],
                                     wrt[:, 128 * j:128 * (j + 1)],
                                     h2aug[:], start=True, stop=True)
                expt = spool.tile([128, NJ], f32, tag="expt")
                rsum = spool.tile([128, 1], f32, tag="rsum")
                nc.scalar.activation(expt[:], lg[:], AF.Exp,
                                     accum_out=rsum[:])
                if f16_out:
                    expt_out = spool.tile([128, NJ], f16, tag="expt16")
                    nc.vector.tensor_copy(expt_out[:], expt[:])
                else:
                    expt_out = expt
                sps = psW.tile([1, 1], f32, tag="sps")
                nc.tensor.matmul(sps[:], rsum[:], ones_col[:],
                                 start=True, stop=True)
                ssb = spool.tile([1, 1], f32, tag="ssb")
                nc.vector.tensor_copy(ssb[:], sps[:])
                nc.sync.dma_start(d_oexp[:], expt_out[:])
                nc.sync.dma_start(d_osum[:], ssb[:])

    nc.compile()
    return nc


def _rep(a):
    """Stack N_CORES copies along axis 0 (global array for P('core'))."""
    return np.ascontiguousarray(
        np.broadcast_to(a[None], (N_CORES,) + a.shape)
        .reshape((N_CORES * a.shape[0],) + a.shape[1:]))


GORD = (0, 1, 3, 2)  # torch (i,f,g,o) -> ours (i,f,o,g)


def _perm1(v):  # [1000,...] -> [1024,...] gate-reordered+padded
    out = np.zeros((1024,) + v.shape[1:], np.float32)
    for k, G in enumerate(GORD):
        out[256 * k:256 * k + 250] = v[250 * G:250 * G + 250]
    return out


def _perm2(v):  # [400,...] -> [400,...] gate-reordered
    return np.concatenate([v[100 * G:100 * G + 100] for G in GORD], 0)


def _prep_weight_tensors(inputs):
    """Host prep of every weight-derived device tensor (global stacked
    layout).  Returns (tensors, b_p_host)."""
    f = lambda k: np.asarray(inputs[k], dtype=np.float32)
    W_s1 = f("W_s1"); b_s1 = f("b_s1")
    W_ih1 = f("W_ih1"); W_hh1 = f("W_hh1")
    b1 = f("b_ih1") + f("b_hh1")
    W_p = f("W_p"); b_p = f("b_p")
    W_ih2 = f("W_ih2"); W_hh2 = f("W_hh2")
    b2 = f("b_ih2") + f("b_hh2")
    W_r = f("W_r"); b_r = f("b_r")

    whh1t = np.ascontiguousarray(_perm1(W_hh1).T)              # [250,1024]
    wih1_cols = np.concatenate([W_ih1 + b1[:, None],
                                b1[:, None]], axis=1)          # [1000,41]
    wih1g = (_perm1(wih1_cols).reshape(8, 128, 41)
             .transpose(1, 2, 0).reshape(128, 41 * 8))
    wih1g = np.ascontiguousarray(wih1g)
    wpt = np.ascontiguousarray(W_p.T)                          # [250,40]
    whh2t = np.ascontiguousarray(_perm2(W_hh2).T)              # [100,400]
    wih2_cols = np.concatenate([W_ih2 + b2[:, None],
                                b2[:, None]], axis=1)          # [400,41]
    wih2g = (_perm2(wih2_cols).reshape(4, 100, 41)
             .transpose(1, 2, 0).reshape(100, 41 * 4))
    wih2g = np.ascontiguousarray(wih2g)
    ws1t = np.ascontiguousarray(W_s1.T)                        # [4096,250]
    bs1 = np.ascontiguousarray(b_s1.reshape(1, 250))
    # W_r^T + bias row, split into per-core column shards, stacked on axis 0
    wrt_full = np.concatenate([W_r.T, b_r[None, :]], 0)        # [101,262144]
    wrt_g = np.ascontiguousarray(
        wrt_full.reshape(101, N_CORES, SHARD).transpose(1, 0, 2)
        .reshape(N_CORES * 101, SHARD))                        # [808,32768]

    tensors = dict(ws1t=_rep(ws1t), bs1=_rep(bs1), whh1t=_rep(whh1t),
                   wih1g=_rep(wih1g), wpt=_rep(wpt), whh2t=_rep(whh2t),
                   wih2g=_rep(wih2g), wrt=wrt_g)
    return tensors, np.array(b_p, copy=True)


def _wr_quick_sig(a):
    """O(0.2ms) signature of W_r: shape/dtype + fixed strided sample."""
    a = np.asarray(a)
    return (a.shape, str(a.dtype), a.reshape(-1)[::1601].copy())


def _wr_full_sum(a):
    """Full-coverage byte checksum (~17ms for 100MB): catches any change."""
    flat = np.asarray(a).reshape(-1)
    if flat.nbytes % 8 == 0:
        return int(flat.view(np.int64).sum(dtype=np.uint64))
    return int(flat.view(np.int32).sum(dtype=np.uint64))


def _wr_quick_eq(s1, s2):
    return (s1[0] == s2[0] and s1[1] == s2[1]
            and np.array_equal(s1[2], s2[2]))


def _get_state():
    if "state" in _CACHE:
        return _CACHE["state"]
    import jax
    from jax.sharding import Mesh, PartitionSpec, NamedSharding
    from jax.experimental.shard_map import shard_map
    import concourse.mybir as mybir
    from concourse import bass2jax

    nc = _build_program()
    bass2jax.install_neuronx_cc_hook()

    partition_name = (nc.partition_id_tensor.name
                      if getattr(nc, "partition_id_tensor", None) is not None
                      else None)
    dbg_name = (nc.dbg_addr.name
                if getattr(nc, "dbg_addr", None) is not None else None)
    in_names, out_names, out_avals = [], [], []
    for alloc in nc.m.functions[0].allocations:
        if not isinstance(alloc, mybir.MemoryLocationSet):
            continue
        name = alloc.memorylocations[0].name
        if alloc.kind == "ExternalInput":
            if name != partition_name:
                in_names.append(name)
        elif alloc.kind == "ExternalOutput":
            out_names.append(name)
            out_avals.append(jax.core.ShapedArray(
                tuple(alloc.tensor_shape), mybir.dt.np(alloc.dtype)))
    n_params = len(in_names)
    bind_names = tuple(list(in_names) + list(out_names)
                       + ([partition_name] if partition_name else []))

    def _gspolicy_v2(*args):
        operands = list(args)
        if partition_name is not None:
            operands.append(bass2jax.partition_id_tensor())
        outs = bass2jax._bass_exec_p.bind(
            *operands,
            out_avals=tuple(out_avals),
            in_names=bind_names,
            out_names=tuple(out_names),
            lowering_input_output_aliases=(),
            sim_require_finite=True,
            sim_require_nnan=True,
            nc=nc,
        )
        return tuple(outs)

    devices = jax.devices()[:N_CORES]
    assert len(devices) == N_CORES
    mesh = Mesh(np.asarray(devices), ("core",))
    P = PartitionSpec
    # per-call activation tensors are replicated (uploaded once, 1/8 the
    # bytes); weights + dead zero-outs are sharded on axis 0
    rep_names = {"xT", "gz"}
    in_specs = tuple(P() if n in rep_names else P("core")
                     for n in in_names) + (P("core"),) * len(out_names)
    fn = jax.jit(
        shard_map(_gspolicy_v2, mesh=mesh, in_specs=in_specs,
                  out_specs=(P("core"),) * len(out_names), check_rep=False),
        keep_unused=True,
    )
    sharding = NamedSharding(mesh, P("core"))
    rep_sharding = NamedSharding(mesh, P())
    # dead inputs: outputs are fully written by the kernel, so the donated-
    # zero trick from run_bass_via_pjrt is unnecessary -- keep permanent
    # device-resident dummies instead of shipping 4 MB of zeros per call.
    zeros_dev = [jax.device_put(
        np.zeros((N_CORES * a.shape[0],) + tuple(a.shape[1:]), a.dtype),
        sharding) for a in out_avals]
    state = dict(jax=jax, nc=nc, fn=fn, in_names=in_names,
                 out_names=out_names, dbg_name=dbg_name,
                 sharding=sharding, rep_sharding=rep_sharding,
                 zeros_dev=zeros_dev)
    _CACHE["state"] = state
    return state


def _verify_weights(wc, inputs):
    """True iff `inputs` weights match the cached device-resident bundle.
    Identity hit is free; content re-verification costs ~18ms (full-coverage
    W_r checksum -- a sampled check alone provably misses single-element
    tampers)."""
    key = tuple(id(inputs[k]) for k in _W_KEYS)
    if wc["key"] == key:
        return True
    same = (all(np.array_equal(np.asarray(inputs[k]), wc["host"][k])
                for k in _W_KEYS if k != "W_r")
            and _wr_quick_eq(_wr_quick_sig(inputs["W_r"]), wc["wr_quick"])
            and _wr_full_sum(inputs["W_r"]) == wc["wr_sum"])
    if same:
        wc["key"] = key
        wc["refs"] = [inputs[k] for k in _W_KEYS]
    return same


def _rebuild_weights(state, inputs):
    jax = state["jax"]
    tensors, b_p = _prep_weight_tensors(inputs)
    dev = {name: jax.device_put(arr, state["sharding"])
           for name, arr in tensors.items()}
    host = {k: np.array(inputs[k], copy=True)
            for k in _W_KEYS if k != "W_r"}
    wc = dict(key=tuple(id(inputs[k]) for k in _W_KEYS),
              dev=dev, b_p=b_p, host=host,
              wr_quick=_wr_quick_sig(inputs["W_r"]),
              wr_sum=_wr_full_sum(inputs["W_r"]),
              refs=[inputs[k] for k in _W_KEYS])
    _CACHE["weights"] = wc
    return wc


def _get_weights(state, inputs):
    """Verified device-resident weight bundle (non-optimistic helper)."""
    wc = _CACHE.get("weights")
    if wc is not None and _verify_weights(wc, inputs):
        return wc
    return _rebuild_weights(state, inputs)


class _Result:
    def __init__(self, results=None):
        self.results = results
        self.exec_time_ns = None
        self.profile_json = None


def run(inputs, trace=False):
    if trace:
        return _run_traced(inputs)
    if _CACHE.get("fast_path_broken"):
        return _run_fallback(inputs)
    try:
        return _run_fast(inputs)
    except Exception:
        # transient axon-tunnel flake -> one retry; persistent failure
        # (e.g. stale staged executable) -> permanent fallback to the
        # robust run_bass_kernel_spmd dispatch path
        try:
            return _run_fast(inputs)
        except Exception:
            _CACHE["fast_path_broken"] = True
            return _run_fallback(inputs)


def _dispatch(state, wc, act_dev):
    jax = state["jax"]
    args = []
    for name in state["in_names"]:
        if name in wc["dev"]:
            args.append(wc["dev"][name])
        elif name in act_dev:
            args.append(act_dev[name])
        elif name == state["dbg_name"]:
            args.append(jax.device_put(
                np.zeros((N_CORES, 2), np.uint32), state["sharding"]))
        else:
            raise KeyError(f"unmapped kernel input {name}")
    args.extend(state["zeros_dev"])
    return state["fn"](*args)


def _run_fast(inputs):
    state = _get_state()
    jax = state["jax"]

    # per-call activations (b_p read straight from inputs so this never
    # depends on the weight-cache state)
    x = np.asarray(inputs["x"], dtype=np.float32)
    gumbel = np.asarray(inputs["gumbel_noise"], dtype=np.float32)
    b_p = np.asarray(inputs["b_p"], dtype=np.float32)
    xT = np.ascontiguousarray(x.reshape(32, 128).T)            # [128,32]
    gz = (gumbel + b_p[None, :]).reshape(1, MAX_LEN * VOCAB)
    dxT, dgz = jax.device_put((xT, np.ascontiguousarray(gz)),
                              state["rep_sharding"])
    act_dev = {"xT": dxT, "gz": dgz}

    # optimistic dispatch: launch with the cached device weights while the
    # ~18ms content verification runs on the host, hidden behind the ~90ms
    # tunnel round trip.  If verification fails, discard the in-flight
    # result and re-run with freshly uploaded weights.
    outs = None
    wc = _CACHE.get("weights")
    if wc is not None:
        outs = _dispatch(state, wc, act_dev)
        if not _verify_weights(wc, inputs):
            outs = None
    if outs is None:
        wc = _rebuild_weights(state, inputs)
        outs = _dispatch(state, wc, act_dev)

    out_map = dict(zip(state["out_names"], outs))
    # Single device->host fetch (out_exp only): the axon tunnel costs a
    # full ~75ms round trip per blocking read, so the softmax denominator
    # is summed host-side in f64 instead of fetching out_s.
    e = np.asarray(out_map["out_exp"])                         # [1024,256] f16
    # per-core block [128, 256]: class = 128*j + p  -> transpose to [256,128].
    # astype before reshape fuses transpose+widen into one copy.
    full = (e.reshape(N_CORES, 128, NJ).transpose(0, 2, 1)
            .astype(np.float32).reshape(-1))
    out = full / np.float32(full.sum(dtype=np.float64))
    return out, _Result()


def _prep_inputs_percore(inputs):
    """Per-core input maps for the traced (run_bass_kernel_spmd) path."""
    tensors, b_p = _prep_weight_tensors(inputs)
    x = np.asarray(inputs["x"], dtype=np.float32)
    gumbel = np.asarray(inputs["gumbel_noise"], dtype=np.float32)
    xT = np.ascontiguousarray(x.reshape(32, 128).T)
    gz = np.ascontiguousarray(
        (gumbel + b_p[None, :]).reshape(1, MAX_LEN * VOCAB))
    maps = []
    for c in range(N_CORES):
        m = {}
        for name, arr in tensors.items():
            d0 = arr.shape[0] // N_CORES
            m[name] = np.ascontiguousarray(arr[d0 * c:d0 * (c + 1)])
        m["xT"] = xT
        m["gz"] = gz
        maps.append(m)
    return maps


def _fallback_nc():
    if "nc_fb" not in _CACHE:
        _CACHE["nc_fb"] = _build_program(f16_out=False)
    return _CACHE["nc_fb"]


def _gather_percore(res):
    parts, total = [], 0.0
    for c in range(N_CORES):
        e = np.asarray(res.results[c]["out_exp"])
        parts.append(e.T.reshape(-1))
        total += float(np.asarray(res.results[c]["out_s"]).reshape(-1)[0])
    full = np.concatenate(parts).astype(np.float64)
    return (full / total).astype(np.float32)


def _run_fallback(inputs):
    from concourse.bass_utils import run_bass_kernel_spmd
    nc = _fallback_nc()
    maps = _prep_inputs_percore(inputs)
    res = run_bass_kernel_spmd(nc, maps, list(range(N_CORES)), trace=False)
    return _gather_percore(res), res


def _run_traced(inputs):
    from concourse.bass_utils import run_bass_kernel_spmd
    nc = _fallback_nc()
    maps = _prep_inputs_percore(inputs)
    res = run_bass_kernel_spmd(nc, maps, list(range(N_CORES)), trace=True)
    return _gather_percore(res), res


def kernel(**inputs):
    out, _ = run(inputs, trace=False)
    return out
